# revision 41
# baseline (speedup 1.0000x reference)
"""Causal single-head attention (B=4, S=2048, E=1024, D=128) on 8 trn2 cores.

Sharding: 2 cores per batch, balanced at 128-row q-tile granularity.

Host ships x^T (E on partitions) with the batch's sixteen 128-row tiles
pair-swapped for role 0 (slot s holds original tile s^1) and natural for
role 1. Under this arrangement both roles' q-tiles land on the same static
slots {1,2,5,6,9,10,13,14} and the causal prefix property holds: program
position i (q-slot QSL[i]) attends key slots [0, 2i+2), and only the last
two key slots need masking. The mask is (col >= scal[p]) with a per-role
host-baked scalar in {p-128, p, p+128} (all-valid / triangle /
all-invalid): 16 tiny Pool-engine ops per core.

Projections run on the fp8 (e4m3) copy of x^T via DoubleRow matmuls
(256-deep contraction, 2x PE rate); weights ship as fp8*16 and the exp
scale absorbs 1/256. V for key-slot 0 is computed in bf16 (rows with a
short causal context consume those values with little averaging; fp8
noise there would break the 2e-2 gate) - everything else averages the
fp8 quantization noise away. Scores/AV/rowsum matmuls run in bf16.

Per key-slot group tb: K^T/Q^T fp8-DR, V^T (mixed) re-transposed via PE;
the next group's projections are emitted between the two attention
positions of the current group so the PE fills the Act-engine exp time.
ot (bf16) and rs (f32) stream out per position pair; host divides and
scatters rows.

PSUM (8 banks): pk(1) pv(1) qpo(1: Q^T cols 0-255 + the two AV accum
regions) tr(1) st(3) rs(1 carved in two). Carved banks rely on the
co-resident accumulation groups' windows never interleaving.
"""

import math

import numpy as np

B, S, E, D = 4, 2048, 1024, 128
P = 128
EC = E // P            # 8 E-chunks
NT = S // P            # 16 key slots
NPOS = 8               # q positions per core
QSL = (1, 2, 5, 6, 9, 10, 13, 14)   # q-slot for position i (both roles)
SCALE = 1.0 / math.sqrt(D)
WSC = 512.0            # fp8 weight scale (clears e4m3 denormals; max|w|*WSC
                       # stays under the 240 e4m3 limit); exp absorbs 1/WSC^2
SCALE8 = SCALE / (WSC * WSC)


def _role_tile(role, slot):
    """Original 128-row tile held at slot `slot` for this role."""
    return slot ^ 1 if role == 0 else slot


def _qtile(role, pos):
    return _role_tile(role, QSL[pos])


def _build_nc():
    from contextlib import ExitStack

    import concourse.bass as bass
    import concourse.tile as tile
    from concourse import bacc, masks, mybir

    bf16 = mybir.dt.bfloat16
    fp16 = mybir.dt.float16
    f32 = mybir.dt.float32
    fp8 = mybir.dt.float8e4
    AF = mybir.ActivationFunctionType
    DR = mybir.MatmulPerfMode.DoubleRow

    nc = bacc.Bacc("TRN2", target_bir_lowering=False, debug=False)

    x8_in = nc.dram_tensor("x8t", [P, 4, 2, S], fp8, kind="ExternalInput")
    xr8_in = nc.dram_tensor("xr8", [P, 4, 2, 2 * P], fp8, kind="ExternalInput")
    w8_in = {
        n: nc.dram_tensor(n, [P, 4, 2, D], fp8, kind="ExternalInput")
        for n in ("wk8", "wq8", "wv8", "wkr8", "wqr8", "wvr8")
    }
    cf_in = nc.dram_tensor("cf32", [P, 3 + 2 * NPOS], f32, kind="ExternalInput")
    ot_out = nc.dram_tensor("ot", [P, NPOS * P], bf16, kind="ExternalOutput")
    rs_out = nc.dram_tensor("rs", [1, NPOS * P], f32, kind="ExternalOutput")

    def mm(out, lhsT, rhs, start, stop):
        nc.tensor.matmul(out, lhsT, rhs, start=start, stop=stop)

    with tile.TileContext(nc) as tc, ExitStack() as ctx:
        consts = ctx.enter_context(tc.tile_pool(name="consts", bufs=1))
        xb_pool = ctx.enter_context(tc.tile_pool(name="xb", bufs=2))
        pt_pool = ctx.enter_context(tc.tile_pool(name="pt", bufs=5))
        out_pool = ctx.enter_context(tc.tile_pool(name="outp", bufs=1))
        vt_pool = ctx.enter_context(tc.tile_pool(name="vt", bufs=2))
        pk_psum = ctx.enter_context(tc.tile_pool(name="pkp", bufs=1, space="PSUM"))
        pv_psum = ctx.enter_context(tc.tile_pool(name="pvp", bufs=1, space="PSUM"))
        qpo_psum = ctx.enter_context(tc.tile_pool(name="qpo", bufs=1, space="PSUM"))
        tr_psum = ctx.enter_context(tc.tile_pool(name="trp", bufs=1, space="PSUM"))
        st_psum = ctx.enter_context(tc.tile_pool(name="stp", bufs=3, space="PSUM"))
        rsb_psum = ctx.enter_context(tc.tile_pool(name="rsb", bufs=1, space="PSUM"))

        # ---- input DMAs ------------------------------------------------
        # DMA_ENGINES serializes transfers in issue order; each weight rides
        # just ahead of the x tensor it gates.
        w_sb = {}
        w_sb["wk8"] = consts.tile([P, 4, 2, D], fp8, name="w_wk8")
        nc.sync.dma_start(out=w_sb["wk8"][:], in_=w8_in["wk8"][:, :, :, :])
        x8_tiles = {}

        def x8_dma(tb, split=False):
            t = xb_pool.tile([P, 4, 2, 512], fp8, tag="x8", bufs=2,
                             name=f"x8_{tb}")
            if split:
                nc.sync.dma_start(
                    out=t[:, 0:2, :, :],
                    in_=x8_in[:, 0:2, :, tb * 512 : (tb + 1) * 512],
                )
                nc.sync.dma_start(
                    out=t[:, 2:4, :, :],
                    in_=x8_in[:, 2:4, :, tb * 512 : (tb + 1) * 512],
                )
            else:
                nc.sync.dma_start(
                    out=t[:], in_=x8_in[:, :, :, tb * 512 : (tb + 1) * 512]
                )
            x8_tiles[tb] = t

        x8_dma(0)
        w_sb["wkr8"] = consts.tile([P, 4, 2, D], fp8, name="w_wkr8")
        nc.sync.dma_start(out=w_sb["wkr8"][:], in_=w8_in["wkr8"][:, :, :, :])
        w_sb["wq8"] = consts.tile([P, 4, 2, D], fp8, name="w_wq8")
        nc.sync.dma_start(out=w_sb["wq8"][:], in_=w8_in["wq8"][:, :, :, :])
        w_sb["wqr8"] = consts.tile([P, 4, 2, D], fp8, name="w_wqr8")
        nc.sync.dma_start(out=w_sb["wqr8"][:], in_=w8_in["wqr8"][:, :, :, :])
        cf = consts.tile([P, 3 + 2 * NPOS], f32, name="cf")
        nc.gpsimd.dma_start(out=cf[:], in_=cf_in[:, :])
        w_sb["wv8"] = consts.tile([P, 4, 2, D], fp8, name="w_wv8")
        nc.sync.dma_start(out=w_sb["wv8"][:], in_=w8_in["wv8"][:, :, :, :])
        w_sb["wvr8"] = consts.tile([P, 4, 2, D], fp8, name="w_wvr8")
        nc.sync.dma_start(out=w_sb["wvr8"][:], in_=w8_in["wvr8"][:, :, :, :])
        xr8 = consts.tile([P, 4, 2, 2 * P], fp8, name="xr8")
        nc.sync.dma_start(out=xr8[:], in_=xr8_in[:, :, :, :])
        x8_dma(1)
        x8_dma(2)
        x8_dma(3)

        # on-chip consts: identity (for PE transpose), column-iota, ones
        ident = consts.tile([P, P], bf16)
        masks.make_identity(nc, ident[:])
        qiota = consts.tile([P, P], fp16)
        nc.gpsimd.iota(qiota[:], pattern=[[1, P]], base=0,
                       channel_multiplier=0,
                       allow_small_or_imprecise_dtypes=True)
        ones_c = consts.tile([P, 1], bf16)
        nc.gpsimd.memset(ones_c[:], 1.0)
        b_sb = {"bk": cf[:, 0:1], "bq": cf[:, 1:2], "bv": cf[:, 2:3]}
        tscal = cf[:, 3:]

        kt_sb = {}   # per-tb K^T [d, 4, t]
        v_sb = {}    # per-tb V natural [t, 4, d]
        qt_sb = {}   # per-tb Q^T [d, 2, q]
        ot_sb = out_pool.tile([P, NPOS * P], bf16)
        rs_sb = out_pool.tile([1, NPOS * P], f32)

        # carved psum banks (groups' accumulation windows never interleave)
        qpo = qpo_psum.tile([P, 512], f32)    # pq: 0-255, po0: 256-383, po1: 384-511
        rsb = rsb_psum.tile([1, 2 * P], f32)  # rs0: cols 0-127, rs1: 128-255

        def phase_a_kq(tb):
            x8 = x8_tiles[tb]
            # K^T via fp8 DoubleRow: 4 matmuls of 256-deep contraction
            pk = pk_psum.tile([P, 4, P], f32, tag="pk", name=f"pk_{tb}")
            for i, (wn, g) in enumerate(
                [(n, g) for n in ("wk8", "wkr8") for g in range(4)]
            ):
                nc.tensor.matmul(
                    pk[:], w_sb[wn][:, g, :, :], x8[:, g, :, :],
                    start=(i == 0), stop=(i == 7), perf_mode=DR,
                )
            kt = consts.tile([P, 4, P], bf16, name=f"kt_{tb}")
            nc.vector.tensor_scalar_add(kt[:], pk[:], b_sb["bk"])
            kt_sb[tb] = kt

            # Q^T for the two q-slots in this tb, fp8 DoubleRow
            for s01 in range(2):
                slot = QSL[2 * tb + s01]
                col = (slot % 4) * P
                for i, (wn, g) in enumerate(
                    [(n, g) for n in ("wq8", "wqr8") for g in range(4)]
                ):
                    nc.tensor.matmul(
                        qpo[:, s01 * P : (s01 + 1) * P],
                        w_sb[wn][:, g, :, :],
                        x8[:, g, :, col : col + P],
                        start=(i == 0), stop=(i == 7), perf_mode=DR,
                    )
            qt = consts.tile([P, 2, P], bf16, name=f"qt_{tb}")
            nc.vector.tensor_scalar_add(qt[:], qpo[:, 0 : 2 * P], b_sb["bq"])
            qt_sb[tb] = qt

        def phase_a_v(tb):
            # V^T: key-slot 0 in bf16 (short-context rows consume it with
            # little averaging), the rest fp8 DoubleRow
            pv = pv_psum.tile([P, 512], f32, tag="pv", name=f"pv_{tb}")
            vt = vt_pool.tile([P, 512], bf16, tag="vt", name=f"vt_{tb}")
            x8 = x8_tiles[tb]
            if tb == 0:
                # slots 0-1 (the short-context rows' keys; orig tile 0 sits
                # at slot 1 under the role-0 swap) get a 4-term compensated
                # fp8 projection - bf16-grade accuracy, all DoubleRow
                for g in range(4):
                    nc.tensor.matmul(
                        pv[:, 2 * P : 512], w_sb["wv8"][:, g, :, :],
                        x8[:, g, :, 2 * P : 512],
                        start=(g == 0), stop=(g == 3), perf_mode=DR,
                    )
                terms = [(x8, "wv8"), (x8, "wvr8"), (xr8, "wv8"), (xr8, "wvr8")]
                n_mm = len(terms) * 4
                for i, (xx, wn) in enumerate(terms):
                    for g in range(4):
                        nc.tensor.matmul(
                            pv[:, 0 : 2 * P], w_sb[wn][:, g, :, :],
                            (xr8[:, g, :, :] if xx is xr8
                             else x8[:, g, :, 0 : 2 * P]),
                            start=(i * 4 + g == 0), stop=(i * 4 + g == n_mm - 1),
                            perf_mode=DR,
                        )
                nc.vector.tensor_scalar(
                    out=vt[:], in0=pv[:],
                    scalar1=1.0 / WSC, scalar2=b_sb["bv"],
                    op0=mybir.AluOpType.mult, op1=mybir.AluOpType.add,
                )
            else:
                for g in range(4):
                    nc.tensor.matmul(
                        pv[:], w_sb["wv8"][:, g, :, :], x8[:, g, :, :],
                        start=(g == 0), stop=(g == 3), perf_mode=DR,
                    )
                nc.vector.tensor_scalar(
                    out=vt[:], in0=pv[:], scalar1=1.0 / WSC, scalar2=b_sb["bv"],
                    op0=mybir.AluOpType.mult, op1=mybir.AluOpType.add,
                )
            tr = tr_psum.tile([P, 4, P], bf16, tag="tr", name=f"tr_{tb}")
            for tt in range(4):
                nc.tensor.matmul(
                    tr[:, tt, :],
                    vt[:, tt * P : (tt + 1) * P],
                    ident[:],
                    is_transpose=True,
                    start=(tt == 0),
                    stop=(tt == 3),
                )
            v = consts.tile([P, 4, P], bf16, name=f"v_{tb}")
            nc.vector.tensor_copy(v[:], tr[:])
            v_sb[tb] = v

        def b_alloc(pos):
            e_n = 2 * pos + 2
            return pt_pool.tile([P, e_n, P], bf16, tag="pt", name=f"pt_{pos}")

        def b_chunks(pos, pt, clo, chi):
            e_n = 2 * pos + 2
            qtb, qs = divmod(pos, 2)
            qt = qt_sb[qtb][:, qs, :]
            nchunks = (e_n + 3) // 4
            for c in range(clo, min(chi, nchunks)):
                j0 = 4 * c
                cs = min(4, e_n - j0)
                st = st_psum.tile([P, cs, P], f32, tag="st", name=f"st_{pos}_{c}")
                for jj in range(cs):
                    j = j0 + jj
                    mm(st[:, jj, :], kt_sb[j // 4][:, j % 4, :], qt, jj == 0, jj == cs - 1)
                nc.scalar.activation(
                    out=pt[:, j0 : j0 + cs, :], in_=st[:, :, :], func=AF.Exp, scale=SCALE8
                )

        def b_masks(pos, pt):
            e_n = 2 * pos + 2
            for jj in range(2):
                j = e_n - 2 + jj
                nc.vector.scalar_tensor_tensor(
                    out=pt[:, j, :],
                    in0=qiota[:],
                    scalar=tscal[:, 2 * pos + jj : 2 * pos + jj + 1],
                    in1=pt[:, j, :],
                    op0=mybir.AluOpType.is_ge,
                    op1=mybir.AluOpType.mult,
                )

        def phase_b_scores(pos):
            pt = b_alloc(pos)
            b_chunks(pos, pt, 0, 4)
            b_masks(pos, pt)
            return pt

        def phase_b_tail(pos, pt):
            e_n = 2 * pos + 2
            po = qpo[:, 256 + (pos % 2) * P : 256 + (pos % 2 + 1) * P]
            rsp = rsb[0:1, (pos % 2) * P : (pos % 2 + 1) * P]
            # masked tail slots last so the mask latency hides behind the
            # unmasked AV/rowsum matmuls
            for jj, j in enumerate(list(range(e_n - 2)) + [e_n - 2, e_n - 1]):
                mm(po, v_sb[j // 4][:, j % 4, :], pt[:, j, :], jj == 0, jj == e_n - 1)
                mm(rsp, ones_c[:], pt[:, j, :], jj == 0, jj == e_n - 1)
            nc.vector.tensor_copy(ot_sb[:, pos * P : (pos + 1) * P], po)
            nc.vector.tensor_copy(rs_sb[0:1, pos * P : (pos + 1) * P], rsp)

        phase_a_kq(0)
        pt_first = phase_b_scores(0)
        phase_a_v(0)
        for tb in range(4):
            pt0 = pt_first if tb == 0 else phase_b_scores(2 * tb)
            if tb < 3:
                phase_a_kq(tb + 1)
            phase_b_tail(2 * tb, pt0)
            if tb == 3:
                nc.sync.dma_start(out=ot_out[:, 6 * P : 7 * P], in_=ot_sb[:, 6 * P : 7 * P])
                nc.scalar.dma_start(out=rs_out[:, 6 * P : 7 * P], in_=rs_sb[0:1, 6 * P : 7 * P])
            pt1 = phase_b_scores(2 * tb + 1)
            if tb < 3:
                phase_a_v(tb + 1)
            phase_b_tail(2 * tb + 1, pt1)
            if tb == 1:
                nc.sync.dma_start(out=ot_out[:, : 4 * P], in_=ot_sb[:, : 4 * P])
                nc.scalar.dma_start(out=rs_out[:, : 4 * P], in_=rs_sb[0:1, : 4 * P])
            if tb == 2:
                nc.sync.dma_start(out=ot_out[:, 4 * P : 6 * P], in_=ot_sb[:, 4 * P : 6 * P])
                nc.scalar.dma_start(out=rs_out[:, 4 * P : 6 * P], in_=rs_sb[0:1, 4 * P : 6 * P])

        nc.sync.dma_start(out=ot_out[:, 7 * P :], in_=ot_sb[:, 7 * P :])
        nc.scalar.dma_start(out=rs_out[:, 7 * P :], in_=rs_sb[0:1, 7 * P :])

    nc.compile()
    return nc


_NC_CACHE = {}


def _get_nc():
    if "nc" not in _NC_CACHE:
        _NC_CACHE["nc"] = _build_nc()
    return _NC_CACHE["nc"]


def _get_runner():
    """Cached PJRT executable (same lowering as bass2jax.run_bass_via_pjrt,
    but the jitted function is built once and reused across calls)."""
    if "runner" in _NC_CACHE:
        return _NC_CACHE["runner"]

    import jax
    from jax.sharding import Mesh, PartitionSpec
    from jax.experimental.shard_map import shard_map
    from concourse import bass2jax, mybir

    nc = _get_nc()
    bass2jax.install_neuronx_cc_hook()

    partition_name = nc.partition_id_tensor.name if nc.partition_id_tensor else None
    in_names, out_names, out_avals = [], [], []
    for alloc in nc.m.functions[0].allocations:
        if not isinstance(alloc, mybir.MemoryLocationSet):
            continue
        name = alloc.memorylocations[0].name
        if alloc.kind == "ExternalInput":
            if name != partition_name:
                in_names.append(name)
        elif alloc.kind == "ExternalOutput":
            out_names.append(name)
            out_avals.append(
                jax.core.ShapedArray(tuple(alloc.tensor_shape), mybir.dt.np(alloc.dtype))
            )
    n_params = len(in_names)
    all_names = in_names + out_names
    if partition_name is not None:
        all_names = all_names + [partition_name]

    def _body(*args):
        operands = list(args)
        if partition_name is not None:
            operands.append(bass2jax.partition_id_tensor())
        outs = bass2jax._bass_exec_p.bind(
            *operands,
            out_avals=tuple(out_avals),
            in_names=tuple(all_names),
            out_names=tuple(out_names),
            lowering_input_output_aliases=(),
            sim_require_finite=True,
            sim_require_nnan=True,
            nc=nc,
        )
        return tuple(outs)

    devices = jax.devices()[:8]
    mesh = Mesh(np.asarray(devices), ("core",))
    n_outs = len(out_names)
    sharded = jax.jit(
        shard_map(
            _body,
            mesh=mesh,
            in_specs=(PartitionSpec("core"),) * (n_params + n_outs),
            out_specs=(PartitionSpec("core"),) * n_outs,
            check_rep=False,
        ),
        donate_argnums=tuple(range(n_params, n_params + n_outs)),
        keep_unused=True,
    )
    runner = {
        "sharded": sharded,
        "in_names": in_names,
        "out_names": out_names,
        "out_avals": out_avals,
    }
    _NC_CACHE["runner"] = runner
    return runner


def _np_dt(name):
    from concourse import mybir

    return mybir.dt.np(getattr(mybir.dt, name))


def _prep_in_concat(x, wq, bq, wk, bk, wv, bv):
    """Per-core inputs, concatenated along axis 0 for shard_map."""
    bf16 = _np_dt("bfloat16")
    fp8 = _np_dt("float8e4")
    x = np.asarray(x, dtype=np.float32)
    wkf = np.asarray(wk, np.float32)
    wvf = np.asarray(wv, np.float32)
    wqf = np.asarray(wq, np.float32)

    # fp8 weights (x WSC): [p, g, t, d] = (w*WSC)[g*256+t*128+p, d]
    def wpack(a):
        return np.ascontiguousarray(
            (a * WSC).reshape(4, 2, P, D).transpose(2, 0, 1, 3)
        )

    wk8 = wpack(wkf).astype(fp8)
    wq8 = wpack(wqf).astype(fp8)
    wv8 = wpack(wvf).astype(fp8)
    # unscaled fp8 residuals: second DoubleRow pass accumulates them directly
    wkr8 = (wpack(wkf) - wk8.astype(np.float32)).astype(fp8)
    wqr8 = (wpack(wqf) - wq8.astype(np.float32)).astype(fp8)
    wvr8 = (wpack(wvf) - wv8.astype(np.float32)).astype(fp8)
    parange = np.arange(P, dtype=np.float32)

    per_core = {n: [] for n in
                ("x8t", "xr8", "wk8", "wq8", "wv8", "wkr8", "wqr8", "wvr8",
                 "cf32")}
    for c in range(8):
        b, role = divmod(c, 2)
        slot2tile = np.array([_role_tile(role, s) for s in range(NT)])
        rows = (slot2tile[:, None] * P + np.arange(P)[None, :]).reshape(S)
        xr = x[b][rows]                       # [S(slot order), E]
        xT = np.ascontiguousarray(xr.T)       # [E, S]
        xTp = np.ascontiguousarray(xT.reshape(4, 2, P, S).transpose(2, 0, 1, 3))
        x8t = xTp.astype(fp8)                 # [p, g, t, s]
        xr8 = (xTp[:, :, :, 0 : 2 * P]
               - x8t[:, :, :, 0 : 2 * P].astype(np.float32)).astype(fp8)
        per_core["x8t"].append(x8t)
        per_core["xr8"].append(xr8)
        per_core["wk8"].append(wk8)
        per_core["wq8"].append(wq8)
        per_core["wv8"].append(wv8)
        per_core["wkr8"].append(wkr8)
        per_core["wqr8"].append(wqr8)
        per_core["wvr8"].append(wvr8)
        cf = np.zeros((P, 3 + 2 * NPOS), dtype=np.float32)
        cf[:, 0] = np.asarray(bk, np.float32) * WSC
        cf[:, 1] = np.asarray(bq, np.float32) * WSC
        cf[:, 2] = np.asarray(bv, np.float32)
        for pos in range(NPOS):
            g = _qtile(role, pos)
            e_n = 2 * pos + 2
            for jj in range(2):
                slot_j = e_n - 2 + jj
                t_tile = _role_tile(role, slot_j)
                cf[:, 3 + 2 * pos + jj] = (t_tile - g) * P + parange
        per_core["cf32"].append(cf)

    runner = _get_runner()
    concat = {n: np.concatenate(v, axis=0) for n, v in per_core.items()}
    return [concat[n] for n in runner["in_names"]]


def _run_concat(concat_in):
    runner = _get_runner()
    zeros = [
        np.zeros((8 * a.shape[0], *a.shape[1:]), a.dtype) for a in runner["out_avals"]
    ]
    out_arrs = runner["sharded"](*concat_in, *zeros)
    ot = np.asarray(out_arrs[runner["out_names"].index("ot")]).astype(np.float32)
    rs = np.asarray(out_arrs[runner["out_names"].index("rs")]).astype(np.float32)
    return ot.reshape(8, P, NPOS * P), rs.reshape(8, NPOS * P)


def _assemble(ot, rs):
    out = np.empty((B, S, D), dtype=np.float32)
    for c in range(8):
        b, role = divmod(c, 2)
        for pos in range(NPOS):
            g = _qtile(role, pos)
            otT = ot[c][:, pos * P : (pos + 1) * P]       # [D, 128]
            rsq = rs[c][pos * P : (pos + 1) * P]          # [128]
            out[b, g * P : (g + 1) * P] = (otT / rsq[None, :]).T
    return out


def kernel(x, wq, bq, wk, bk, wv, bv):
    concat_in = _prep_in_concat(x, wq, bq, wk, bk, wv, bv)
    ot, rs = _run_concat(concat_in)
    return _assemble(ot, rs)


def bench(x, wq, bq, wk, bk, wv, bv, iters=20):
    """Per-launch wall time with device-resident inputs (upper bound on HW exec)."""
    import time

    import jax

    runner = _get_runner()
    concat_in = _prep_in_concat(x, wq, bq, wk, bk, wv, bv)
    dev_in = [jax.device_put(a) for a in concat_in]
    for a in dev_in:
        a.block_until_ready()
    times = []
    for _ in range(iters):
        zeros = [
            np.zeros((8 * a.shape[0], *a.shape[1:]), a.dtype)
            for a in runner["out_avals"]
        ]
        t0 = time.perf_counter()
        out = runner["sharded"](*dev_in, *zeros)
        for a in out:
            a.block_until_ready()
        times.append(time.perf_counter() - t0)
    return times


# revision 43
# speedup vs baseline: 1.0395x; 1.0395x over previous
"""Causal single-head attention (B=4, S=2048, E=1024, D=128) on 8 trn2 cores.

Sharding: 2 cores per batch, balanced at 128-row q-tile granularity.

Host ships x^T (E on partitions) with the batch's sixteen 128-row tiles
pair-swapped for role 0 (slot s holds original tile s^1) and natural for
role 1. Under this arrangement both roles' q-tiles land on the same static
slots {1,2,5,6,9,10,13,14} and the causal prefix property holds: program
position i (q-slot QSL[i]) attends key slots [0, 2i+2), and only the last
two key slots need masking. The mask is (col >= scal[p]) with a per-role
host-baked scalar in {p-128, p, p+128} (all-valid / triangle /
all-invalid): 16 tiny Pool-engine ops per core.

Projections run on the fp8 (e4m3) copy of x^T via DoubleRow matmuls
(256-deep contraction, 2x PE rate); weights ship as fp8*16 and the exp
scale absorbs 1/256. V for key-slot 0 is computed in bf16 (rows with a
short causal context consume those values with little averaging; fp8
noise there would break the 2e-2 gate) - everything else averages the
fp8 quantization noise away. Scores/AV/rowsum matmuls run in bf16.

Per key-slot group tb: K^T/Q^T fp8-DR, V^T (mixed) re-transposed via PE;
the next group's projections are emitted between the two attention
positions of the current group so the PE fills the Act-engine exp time.
ot (bf16) and rs (f32) stream out per position pair; host divides and
scatters rows.

PSUM (8 banks): pk(1) pv(1) qpo(1: Q^T cols 0-255 + the two AV accum
regions) tr(1) st(3) rs(1 carved in two). Carved banks rely on the
co-resident accumulation groups' windows never interleaving.
"""

import math

import numpy as np

B, S, E, D = 4, 2048, 1024, 128
P = 128
EC = E // P            # 8 E-chunks
NT = S // P            # 16 key slots
NPOS = 8               # q positions per core
QSL = (1, 2, 5, 6, 9, 10, 13, 14)   # q-slot for position i (both roles)
SCALE = 1.0 / math.sqrt(D)
WSC = 512.0            # fp8 weight scale (clears e4m3 denormals; max|w|*WSC
                       # stays under the 240 e4m3 limit); exp absorbs 1/WSC^2
SCALE8 = SCALE / (WSC * WSC)


def _role_tile(role, slot):
    """Original 128-row tile held at slot `slot` for this role."""
    return slot ^ 1 if role == 0 else slot


def _qtile(role, pos):
    return _role_tile(role, QSL[pos])


def _build_nc():
    from contextlib import ExitStack

    import concourse.bass as bass
    import concourse.tile as tile
    from concourse import bacc, masks, mybir

    bf16 = mybir.dt.bfloat16
    fp16 = mybir.dt.float16
    f32 = mybir.dt.float32
    fp8 = mybir.dt.float8e4
    AF = mybir.ActivationFunctionType
    DR = mybir.MatmulPerfMode.DoubleRow

    nc = bacc.Bacc("TRN2", target_bir_lowering=False, debug=False)

    x8_in = nc.dram_tensor("x8t", [P, 4, 2, S], fp8, kind="ExternalInput")
    xr8_in = nc.dram_tensor("xr8", [P, 4, 2, 2 * P], fp8, kind="ExternalInput")
    w8_in = {
        n: nc.dram_tensor(n, [P, 4, 2, D], fp8, kind="ExternalInput")
        for n in ("wk8", "wq8", "wv8", "wkr8", "wqr8", "wvr8")
    }
    cf_in = nc.dram_tensor("cf32", [P, 3 + 2 * NPOS], f32, kind="ExternalInput")
    ot_out = nc.dram_tensor("ot", [P, NPOS * P], bf16, kind="ExternalOutput")
    rs_out = nc.dram_tensor("rs", [1, NPOS * P], f32, kind="ExternalOutput")

    def mm(out, lhsT, rhs, start, stop):
        nc.tensor.matmul(out, lhsT, rhs, start=start, stop=stop)

    with tile.TileContext(nc) as tc, ExitStack() as ctx:
        consts = ctx.enter_context(tc.tile_pool(name="consts", bufs=1))
        xb_pool = ctx.enter_context(tc.tile_pool(name="xb", bufs=2))
        pt_pool = ctx.enter_context(tc.tile_pool(name="pt", bufs=5))
        out_pool = ctx.enter_context(tc.tile_pool(name="outp", bufs=1))
        vt_pool = ctx.enter_context(tc.tile_pool(name="vt", bufs=2))
        pk_psum = ctx.enter_context(tc.tile_pool(name="pkp", bufs=1, space="PSUM"))
        pv_psum = ctx.enter_context(tc.tile_pool(name="pvp", bufs=1, space="PSUM"))
        qpo_psum = ctx.enter_context(tc.tile_pool(name="qpo", bufs=1, space="PSUM"))
        tr_psum = ctx.enter_context(tc.tile_pool(name="trp", bufs=1, space="PSUM"))
        st_psum = ctx.enter_context(tc.tile_pool(name="stp", bufs=3, space="PSUM"))
        rsb_psum = ctx.enter_context(tc.tile_pool(name="rsb", bufs=1, space="PSUM"))

        # ---- input DMAs ------------------------------------------------
        # DMA_ENGINES serializes transfers in issue order; each weight rides
        # just ahead of the x tensor it gates.
        w_sb = {}
        w_sb["wk8"] = consts.tile([P, 4, 2, D], fp8, name="w_wk8")
        nc.sync.dma_start(out=w_sb["wk8"][:], in_=w8_in["wk8"][:, :, :, :])
        x8_tiles = {}

        def x8_dma(tb, split=False):
            t = xb_pool.tile([P, 4, 2, 512], fp8, tag="x8", bufs=2,
                             name=f"x8_{tb}")
            if split:
                nc.sync.dma_start(
                    out=t[:, 0:2, :, :],
                    in_=x8_in[:, 0:2, :, tb * 512 : (tb + 1) * 512],
                )
                nc.sync.dma_start(
                    out=t[:, 2:4, :, :],
                    in_=x8_in[:, 2:4, :, tb * 512 : (tb + 1) * 512],
                )
            else:
                nc.sync.dma_start(
                    out=t[:], in_=x8_in[:, :, :, tb * 512 : (tb + 1) * 512]
                )
            x8_tiles[tb] = t

        x8_dma(0)
        w_sb["wkr8"] = consts.tile([P, 4, 2, D], fp8, name="w_wkr8")
        nc.sync.dma_start(out=w_sb["wkr8"][:], in_=w8_in["wkr8"][:, :, :, :])
        w_sb["wq8"] = consts.tile([P, 4, 2, D], fp8, name="w_wq8")
        nc.sync.dma_start(out=w_sb["wq8"][:], in_=w8_in["wq8"][:, :, :, :])
        w_sb["wqr8"] = consts.tile([P, 4, 2, D], fp8, name="w_wqr8")
        nc.sync.dma_start(out=w_sb["wqr8"][:], in_=w8_in["wqr8"][:, :, :, :])
        cf = consts.tile([P, 3 + 2 * NPOS], f32, name="cf")
        nc.gpsimd.dma_start(out=cf[:], in_=cf_in[:, :])
        w_sb["wv8"] = consts.tile([P, 4, 2, D], fp8, name="w_wv8")
        nc.sync.dma_start(out=w_sb["wv8"][:], in_=w8_in["wv8"][:, :, :, :])
        w_sb["wvr8"] = consts.tile([P, 4, 2, D], fp8, name="w_wvr8")
        nc.sync.dma_start(out=w_sb["wvr8"][:], in_=w8_in["wvr8"][:, :, :, :])
        xr8 = consts.tile([P, 4, 2, 2 * P], fp8, name="xr8")
        nc.sync.dma_start(out=xr8[:], in_=xr8_in[:, :, :, :])
        x8_dma(1)
        x8_dma(2)
        x8_dma(3)

        # on-chip consts: identity (for PE transpose), column-iota, ones
        ident = consts.tile([P, P], bf16)
        masks.make_identity(nc, ident[:])
        qiota = consts.tile([P, P], fp16)
        nc.gpsimd.iota(qiota[:], pattern=[[1, P]], base=0,
                       channel_multiplier=0,
                       allow_small_or_imprecise_dtypes=True)
        ones_b = consts.tile([P, P], bf16)
        nc.gpsimd.memset(ones_b[:], 1.0)
        ones8 = consts.tile([P, 2, P], fp8)
        nc.gpsimd.memset(ones8[:], 1.0)
        b_sb = {"bk": cf[:, 0:1], "bq": cf[:, 1:2], "bv": cf[:, 2:3]}
        tscal = cf[:, 3:]

        kt_sb = {}   # per-tb K^T [d, 4, t]
        v_sb = {}    # per-tb V natural [t, 4, d]
        qt_sb = {}   # per-tb Q^T [d, 2, q]
        ot_sb = out_pool.tile([P, NPOS * P], bf16)
        rs_sb = out_pool.tile([1, NPOS * P], f32)

        # carved psum banks (groups' accumulation windows never interleave)
        qpo = qpo_psum.tile([P, 512], f32)    # pq: 0-255, po0: 256-383, po1: 384-511
        rsb = rsb_psum.tile([P, 2 * P], f32)  # rs0: cols 0-127, rs1: 128-255

        def phase_a_kq(tb):
            x8 = x8_tiles[tb]
            # K^T via fp8 DoubleRow: 4 matmuls of 256-deep contraction
            pk = pk_psum.tile([P, 4, P], f32, tag="pk", name=f"pk_{tb}")
            for i, (wn, g) in enumerate(
                [(n, g) for n in ("wk8", "wkr8") for g in range(4)]
            ):
                nc.tensor.matmul(
                    pk[:], w_sb[wn][:, g, :, :], x8[:, g, :, :],
                    start=(i == 0), stop=(i == 7), perf_mode=DR,
                )
            kt = consts.tile([P, 4, P], bf16, name=f"kt_{tb}")
            nc.vector.tensor_scalar_add(kt[:], pk[:], b_sb["bk"])
            kt_sb[tb] = kt

            # Q^T for the two q-slots in this tb, fp8 DoubleRow
            for s01 in range(2):
                slot = QSL[2 * tb + s01]
                col = (slot % 4) * P
                for i, (wn, g) in enumerate(
                    [(n, g) for n in ("wq8", "wqr8") for g in range(4)]
                ):
                    nc.tensor.matmul(
                        qpo[:, s01 * P : (s01 + 1) * P],
                        w_sb[wn][:, g, :, :],
                        x8[:, g, :, col : col + P],
                        start=(i == 0), stop=(i == 7), perf_mode=DR,
                    )
            qt = consts.tile([P, 2, P], bf16, name=f"qt_{tb}")
            nc.vector.tensor_scalar_add(qt[:], qpo[:, 0 : 2 * P], b_sb["bq"])
            qt_sb[tb] = qt

        def phase_a_v(tb):
            # V^T: key-slot 0 in bf16 (short-context rows consume it with
            # little averaging), the rest fp8 DoubleRow
            pv = pv_psum.tile([P, 512], f32, tag="pv", name=f"pv_{tb}")
            vt = vt_pool.tile([P, 512], bf16, tag="vt", name=f"vt_{tb}")
            x8 = x8_tiles[tb]
            if tb == 0:
                # slots 0-1 (the short-context rows' keys; orig tile 0 sits
                # at slot 1 under the role-0 swap) get a 4-term compensated
                # fp8 projection - bf16-grade accuracy, all DoubleRow
                for g in range(4):
                    nc.tensor.matmul(
                        pv[:, 2 * P : 512], w_sb["wv8"][:, g, :, :],
                        x8[:, g, :, 2 * P : 512],
                        start=(g == 0), stop=(g == 3), perf_mode=DR,
                    )
                terms = [(x8, "wv8"), (x8, "wvr8"), (xr8, "wv8"), (xr8, "wvr8")]
                n_mm = len(terms) * 4
                for i, (xx, wn) in enumerate(terms):
                    for g in range(4):
                        nc.tensor.matmul(
                            pv[:, 0 : 2 * P], w_sb[wn][:, g, :, :],
                            (xr8[:, g, :, :] if xx is xr8
                             else x8[:, g, :, 0 : 2 * P]),
                            start=(i * 4 + g == 0), stop=(i * 4 + g == n_mm - 1),
                            perf_mode=DR,
                        )
                nc.vector.tensor_scalar(
                    out=vt[:], in0=pv[:],
                    scalar1=1.0 / WSC, scalar2=b_sb["bv"],
                    op0=mybir.AluOpType.mult, op1=mybir.AluOpType.add,
                )
            else:
                for g in range(4):
                    nc.tensor.matmul(
                        pv[:], w_sb["wv8"][:, g, :, :], x8[:, g, :, :],
                        start=(g == 0), stop=(g == 3), perf_mode=DR,
                    )
                nc.vector.tensor_scalar(
                    out=vt[:], in0=pv[:], scalar1=1.0 / WSC, scalar2=b_sb["bv"],
                    op0=mybir.AluOpType.mult, op1=mybir.AluOpType.add,
                )
            tr = tr_psum.tile([P, 4, P], bf16, tag="tr", name=f"tr_{tb}")
            for tt in range(4):
                nc.tensor.matmul(
                    tr[:, tt, :],
                    vt[:, tt * P : (tt + 1) * P],
                    ident[:],
                    is_transpose=True,
                    start=(tt == 0),
                    stop=(tt == 3),
                )
            v = consts.tile([P, 4, P], bf16 if tb == 0 else fp8, name=f"v_{tb}")
            nc.vector.tensor_copy(v[:], tr[:])
            v_sb[tb] = v

        def b_alloc(pos):
            e_n = 2 * pos + 2
            ptb = pt_pool.tile([P, min(e_n, 4), P], bf16, tag="pt",
                               name=f"pt_{pos}")
            pt8 = None
            if e_n > 4:
                pt8 = pt_pool.tile([P, e_n - 4, P], fp8, tag="pt8", bufs=3,
                                   name=f"pt8_{pos}")
            return (ptb, pt8)

        def b_chunks(pos, pts, clo, chi):
            ptb, pt8 = pts
            e_n = 2 * pos + 2
            qtb, qs = divmod(pos, 2)
            qt = qt_sb[qtb][:, qs, :]
            nchunks = (e_n + 3) // 4
            for c in range(clo, min(chi, nchunks)):
                j0 = 4 * c
                cs = min(4, e_n - j0)
                st = st_psum.tile([P, cs, P], f32, tag="st", name=f"st_{pos}_{c}")
                for jj in range(cs):
                    j = j0 + jj
                    mm(st[:, jj, :], kt_sb[j // 4][:, j % 4, :], qt, jj == 0, jj == cs - 1)
                dst = (ptb[:, j0 : j0 + cs, :] if c == 0
                       else pt8[:, j0 - 4 : j0 - 4 + cs, :])
                nc.scalar.activation(
                    out=dst, in_=st[:, :, :], func=AF.Exp, scale=SCALE8
                )

        def b_masks(pos, pts):
            ptb, pt8 = pts
            e_n = 2 * pos + 2
            for jj in range(2):
                j = e_n - 2 + jj
                sl = ptb[:, j, :] if j < 4 else pt8[:, j - 4, :]
                nc.vector.scalar_tensor_tensor(
                    out=sl,
                    in0=qiota[:],
                    scalar=tscal[:, 2 * pos + jj : 2 * pos + jj + 1],
                    in1=sl,
                    op0=mybir.AluOpType.is_ge,
                    op1=mybir.AluOpType.mult,
                )

        def phase_b_scores(pos):
            pts = b_alloc(pos)
            b_chunks(pos, pts, 0, 4)
            b_masks(pos, pts)
            return pts

        def phase_b_tail(pos, pts):
            ptb, pt8 = pts
            e_n = 2 * pos + 2
            po = qpo[:, 256 + (pos % 2) * P : 256 + (pos % 2 + 1) * P]
            rsp = rsb[:, (pos % 2) * P : (pos % 2 + 1) * P]
            if e_n <= 4:
                # all-bf16 singles; masked tail slots last to hide DVE latency
                for jj, j in enumerate(list(range(e_n - 2)) + [e_n - 2, e_n - 1]):
                    mm(po, v_sb[0][:, j, :], ptb[:, j, :], jj == 0, jj == e_n - 1)
                    mm(rsp, ones_b[:], ptb[:, j, :], jj == 0, jj == e_n - 1)
            else:
                n_pair = (e_n - 4) // 2
                for j in range(4):
                    mm(po, v_sb[0][:, j, :], ptb[:, j, :], j == 0, False)
                    mm(rsp, ones_b[:], ptb[:, j, :], j == 0, False)
                # slot pairs (4,5),(6,7),... - the masked pair goes last
                for k in range(n_pair):
                    j = 4 + 2 * k
                    last = k == n_pair - 1
                    nc.tensor.matmul(
                        po, v_sb[j // 4][:, j % 4 : j % 4 + 2, :],
                        pt8[:, j - 4 : j - 2, :],
                        start=False, stop=last, perf_mode=DR,
                    )
                    nc.tensor.matmul(
                        rsp, ones8[:], pt8[:, j - 4 : j - 2, :],
                        start=False, stop=last, perf_mode=DR,
                    )
            nc.vector.tensor_copy(ot_sb[:, pos * P : (pos + 1) * P], po)
            nc.vector.tensor_copy(rs_sb[0:1, pos * P : (pos + 1) * P], rsp[0:1, :])

        phase_a_kq(0)
        pt_first = phase_b_scores(0)
        phase_a_v(0)
        for tb in range(4):
            pt0 = pt_first if tb == 0 else phase_b_scores(2 * tb)
            if tb < 3:
                phase_a_kq(tb + 1)
            phase_b_tail(2 * tb, pt0)
            if tb == 3:
                nc.sync.dma_start(out=ot_out[:, 6 * P : 7 * P], in_=ot_sb[:, 6 * P : 7 * P])
                nc.scalar.dma_start(out=rs_out[:, 6 * P : 7 * P], in_=rs_sb[0:1, 6 * P : 7 * P])
            pt1 = phase_b_scores(2 * tb + 1)
            if tb < 3:
                phase_a_v(tb + 1)
            phase_b_tail(2 * tb + 1, pt1)
            if tb == 1:
                nc.sync.dma_start(out=ot_out[:, : 4 * P], in_=ot_sb[:, : 4 * P])
                nc.scalar.dma_start(out=rs_out[:, : 4 * P], in_=rs_sb[0:1, : 4 * P])
            if tb == 2:
                nc.sync.dma_start(out=ot_out[:, 4 * P : 6 * P], in_=ot_sb[:, 4 * P : 6 * P])
                nc.scalar.dma_start(out=rs_out[:, 4 * P : 6 * P], in_=rs_sb[0:1, 4 * P : 6 * P])

        nc.sync.dma_start(out=ot_out[:, 7 * P :], in_=ot_sb[:, 7 * P :])
        nc.scalar.dma_start(out=rs_out[:, 7 * P :], in_=rs_sb[0:1, 7 * P :])

    nc.compile()
    return nc


_NC_CACHE = {}


def _get_nc():
    if "nc" not in _NC_CACHE:
        _NC_CACHE["nc"] = _build_nc()
    return _NC_CACHE["nc"]


def _get_runner():
    """Cached PJRT executable (same lowering as bass2jax.run_bass_via_pjrt,
    but the jitted function is built once and reused across calls)."""
    if "runner" in _NC_CACHE:
        return _NC_CACHE["runner"]

    import jax
    from jax.sharding import Mesh, PartitionSpec
    from jax.experimental.shard_map import shard_map
    from concourse import bass2jax, mybir

    nc = _get_nc()
    bass2jax.install_neuronx_cc_hook()

    partition_name = nc.partition_id_tensor.name if nc.partition_id_tensor else None
    in_names, out_names, out_avals = [], [], []
    for alloc in nc.m.functions[0].allocations:
        if not isinstance(alloc, mybir.MemoryLocationSet):
            continue
        name = alloc.memorylocations[0].name
        if alloc.kind == "ExternalInput":
            if name != partition_name:
                in_names.append(name)
        elif alloc.kind == "ExternalOutput":
            out_names.append(name)
            out_avals.append(
                jax.core.ShapedArray(tuple(alloc.tensor_shape), mybir.dt.np(alloc.dtype))
            )
    n_params = len(in_names)
    all_names = in_names + out_names
    if partition_name is not None:
        all_names = all_names + [partition_name]

    def _body(*args):
        operands = list(args)
        if partition_name is not None:
            operands.append(bass2jax.partition_id_tensor())
        outs = bass2jax._bass_exec_p.bind(
            *operands,
            out_avals=tuple(out_avals),
            in_names=tuple(all_names),
            out_names=tuple(out_names),
            lowering_input_output_aliases=(),
            sim_require_finite=True,
            sim_require_nnan=True,
            nc=nc,
        )
        return tuple(outs)

    devices = jax.devices()[:8]
    mesh = Mesh(np.asarray(devices), ("core",))
    n_outs = len(out_names)
    sharded = jax.jit(
        shard_map(
            _body,
            mesh=mesh,
            in_specs=(PartitionSpec("core"),) * (n_params + n_outs),
            out_specs=(PartitionSpec("core"),) * n_outs,
            check_rep=False,
        ),
        donate_argnums=tuple(range(n_params, n_params + n_outs)),
        keep_unused=True,
    )
    runner = {
        "sharded": sharded,
        "in_names": in_names,
        "out_names": out_names,
        "out_avals": out_avals,
    }
    _NC_CACHE["runner"] = runner
    return runner


def _np_dt(name):
    from concourse import mybir

    return mybir.dt.np(getattr(mybir.dt, name))


def _prep_in_concat(x, wq, bq, wk, bk, wv, bv):
    """Per-core inputs, concatenated along axis 0 for shard_map."""
    bf16 = _np_dt("bfloat16")
    fp8 = _np_dt("float8e4")
    x = np.asarray(x, dtype=np.float32)
    wkf = np.asarray(wk, np.float32)
    wvf = np.asarray(wv, np.float32)
    wqf = np.asarray(wq, np.float32)

    # fp8 weights (x WSC): [p, g, t, d] = (w*WSC)[g*256+t*128+p, d]
    def wpack(a):
        return np.ascontiguousarray(
            (a * WSC).reshape(4, 2, P, D).transpose(2, 0, 1, 3)
        )

    wk8 = wpack(wkf).astype(fp8)
    wq8 = wpack(wqf).astype(fp8)
    wv8 = wpack(wvf).astype(fp8)
    # unscaled fp8 residuals: second DoubleRow pass accumulates them directly
    wkr8 = (wpack(wkf) - wk8.astype(np.float32)).astype(fp8)
    wqr8 = (wpack(wqf) - wq8.astype(np.float32)).astype(fp8)
    wvr8 = (wpack(wvf) - wv8.astype(np.float32)).astype(fp8)
    parange = np.arange(P, dtype=np.float32)

    per_core = {n: [] for n in
                ("x8t", "xr8", "wk8", "wq8", "wv8", "wkr8", "wqr8", "wvr8",
                 "cf32")}
    for c in range(8):
        b, role = divmod(c, 2)
        slot2tile = np.array([_role_tile(role, s) for s in range(NT)])
        rows = (slot2tile[:, None] * P + np.arange(P)[None, :]).reshape(S)
        xr = x[b][rows]                       # [S(slot order), E]
        xT = np.ascontiguousarray(xr.T)       # [E, S]
        xTp = np.ascontiguousarray(xT.reshape(4, 2, P, S).transpose(2, 0, 1, 3))
        x8t = xTp.astype(fp8)                 # [p, g, t, s]
        xr8 = (xTp[:, :, :, 0 : 2 * P]
               - x8t[:, :, :, 0 : 2 * P].astype(np.float32)).astype(fp8)
        per_core["x8t"].append(x8t)
        per_core["xr8"].append(xr8)
        per_core["wk8"].append(wk8)
        per_core["wq8"].append(wq8)
        per_core["wv8"].append(wv8)
        per_core["wkr8"].append(wkr8)
        per_core["wqr8"].append(wqr8)
        per_core["wvr8"].append(wvr8)
        cf = np.zeros((P, 3 + 2 * NPOS), dtype=np.float32)
        cf[:, 0] = np.asarray(bk, np.float32) * WSC
        cf[:, 1] = np.asarray(bq, np.float32) * WSC
        cf[:, 2] = np.asarray(bv, np.float32)
        for pos in range(NPOS):
            g = _qtile(role, pos)
            e_n = 2 * pos + 2
            for jj in range(2):
                slot_j = e_n - 2 + jj
                t_tile = _role_tile(role, slot_j)
                cf[:, 3 + 2 * pos + jj] = (t_tile - g) * P + parange
        per_core["cf32"].append(cf)

    runner = _get_runner()
    concat = {n: np.concatenate(v, axis=0) for n, v in per_core.items()}
    return [concat[n] for n in runner["in_names"]]


def _run_concat(concat_in):
    runner = _get_runner()
    zeros = [
        np.zeros((8 * a.shape[0], *a.shape[1:]), a.dtype) for a in runner["out_avals"]
    ]
    out_arrs = runner["sharded"](*concat_in, *zeros)
    ot = np.asarray(out_arrs[runner["out_names"].index("ot")]).astype(np.float32)
    rs = np.asarray(out_arrs[runner["out_names"].index("rs")]).astype(np.float32)
    return ot.reshape(8, P, NPOS * P), rs.reshape(8, NPOS * P)


def _assemble(ot, rs):
    out = np.empty((B, S, D), dtype=np.float32)
    for c in range(8):
        b, role = divmod(c, 2)
        for pos in range(NPOS):
            g = _qtile(role, pos)
            otT = ot[c][:, pos * P : (pos + 1) * P]       # [D, 128]
            rsq = rs[c][pos * P : (pos + 1) * P]          # [128]
            out[b, g * P : (g + 1) * P] = (otT / rsq[None, :]).T
    return out


def kernel(x, wq, bq, wk, bk, wv, bv):
    concat_in = _prep_in_concat(x, wq, bq, wk, bk, wv, bv)
    ot, rs = _run_concat(concat_in)
    return _assemble(ot, rs)


def bench(x, wq, bq, wk, bk, wv, bv, iters=20):
    """Per-launch wall time with device-resident inputs (upper bound on HW exec)."""
    import time

    import jax

    runner = _get_runner()
    concat_in = _prep_in_concat(x, wq, bq, wk, bk, wv, bv)
    dev_in = [jax.device_put(a) for a in concat_in]
    for a in dev_in:
        a.block_until_ready()
    times = []
    for _ in range(iters):
        zeros = [
            np.zeros((8 * a.shape[0], *a.shape[1:]), a.dtype)
            for a in runner["out_avals"]
        ]
        t0 = time.perf_counter()
        out = runner["sharded"](*dev_in, *zeros)
        for a in out:
            a.block_until_ready()
        times.append(time.perf_counter() - t0)
    return times
